# revision 12
# baseline (speedup 1.0000x reference)
"""Multi-head attention Bass kernel for Trainium2, sharded over 8 NeuronCores.

Problem: x [2, 2048, 1024] fp32; W_qkv [3072, 1024]; b_qkv [3072].
  qkv = x @ W_qkv.T + b_qkv ; split into Q,K,V of 8 heads x 128 dims;
  out  = softmax(Q K^T / sqrt(128)) V, heads re-concatenated -> [2, 2048, 1024].

Sharding: 16 (batch, head) pairs over 8 cores -> each core owns one batch
slice (b = core//4) and two heads (h0 = 2*(core%4), h0+1). Each core gets
its batch's x slice [2048, 1024] plus the W^T/bias columns for its heads,
computes the projection and full non-causal attention for its two heads,
and returns [2048, 256] (the two heads' output columns). No collectives.

Kernel internals (per core):
 - x is cast to bf16 and bounced through DRAM so the DMA-transpose engine
   can produce x^T (contraction dim on partitions) for the projection.
 - Q^T, K^T produced directly in [dh, tok] layout (bias added during the
   PSUM->SBUF copy); V in natural [tok, dh] layout with a ones column
   appended so the P@V matmul also produces softmax denominators.
 - Scores are computed transposed (S^T tiles [k, q]) so exp(S^T) is
   directly the lhsT for the P@V matmul -- no on-chip transposes of the
   4M-element attention matrix. exp is computed without max subtraction
   (scores are O(1) here; exp is safely in fp32/bf16 range).
 - Final output = (P@V) * 1/denominator, written as fp32.
"""

import math
from contextlib import ExitStack

import numpy as np

import concourse.bass as bass
import concourse.tile as tile
from concourse import bacc, mybir
from concourse.bass_utils import run_bass_kernel_spmd

# Problem constants (hardcoded per the harness contract).
B = 2
S = 2048
D = 1024
H = 8
DH = 128
N_CORES = 8
HPC = 2  # heads per core
SC = S  # tokens per core (one full batch element)
WCOLS = 3 * HPC * DH  # 768: [q0 q1 k0 k1 v0 v1] blocks of 128
SCALE = 1.0 / math.sqrt(DH)

F32 = mybir.dt.float32
BF16 = mybir.dt.bfloat16

N_CHUNK = 4  # token chunks for the x-transpose pipeline
CHUNK = SC // N_CHUNK  # 512
KO = D // 128  # 8 contraction chunks


def _mha_body(ctx: ExitStack, tc: tile.TileContext, out, x, wt, bias):
    nc = tc.nc

    consts = ctx.enter_context(tc.tile_pool(name="consts", bufs=1))
    xstage = ctx.enter_context(tc.tile_pool(name="xstage", bufs=2))
    xtp = ctx.enter_context(tc.tile_pool(name="xtp", bufs=1))
    qkvp = ctx.enter_context(tc.tile_pool(name="qkvp", bufs=1))

    # ---- constants: W^T (bf16) and biases ----
    wt_f32 = xstage.tile([128, KO, WCOLS], F32, tag="wtf32")
    nc.sync.dma_start(wt_f32, wt.rearrange("(ko ki) m -> ki ko m", ki=128))
    wt_sb = consts.tile([128, KO, WCOLS], BF16)
    nc.vector.tensor_copy(wt_sb, wt_f32)

    # per-partition bias tiles for Q^T / K^T copies: q_h at h*128, k_h at 256+h*128
    bqk = []
    for i in range(2 * HPC):  # q0 q1 k0 k1
        bt = consts.tile([128, 1], F32, tag=f"bqk{i}")
        nc.sync.dma_start(bt, bias[i * 128:(i + 1) * 128].rearrange("(p o) -> p o", o=1))
        bqk.append(bt)
    # V bias replicated across partitions [128, 256]
    bv_rep = consts.tile([128, HPC * DH], F32)
    nc.gpsimd.dma_start(bv_rep, bias[2 * HPC * DH:][None, :].to_broadcast([128, HPC * DH]))

    # ---- x (bf16 from host): DMA-transpose straight from DRAM into x^T chunks ----
    # xT chunks: [128 d_inner, KO d_outer, CHUNK tokens] bf16
    xt = [xtp.tile([128, KO, CHUNK], BF16, tag=f"xt{c}", name=f"xt{c}") for c in range(N_CHUNK)]
    for c in range(N_CHUNK):
        for ko in range(KO):
            nc.sync.dma_start(
                xt[c][:, ko, :],
                x[c * CHUNK:(c + 1) * CHUNK, ko * 128:(ko + 1) * 128],
                transpose=True,
            )

    # ---- persistent QKV tiles ----
    qT = qkvp.tile([128, HPC, SC], BF16, tag="qT")  # [dh, h, tok]
    kT = qkvp.tile([128, HPC, SC], BF16, tag="kT")
    v_sb = qkvp.tile([128, HPC, SC // 128, DH + 1], BF16, tag="v")  # [tok_i, h, tok_o, dh+1]
    nc.vector.memset(v_sb[:, :, :, DH:DH + 1], 1.0)

    # ---- projection ----
    with tc.tile_pool(name="proj_ps", bufs=3, space="PSUM") as proj_ps:
        # Q^T and K^T: out [128 dh, tok], lhsT = W^T chunk, rhs = x^T chunk
        for h in range(HPC):
            for qk in range(2):
                dst = qT if qk == 0 else kT
                col = qk * HPC * DH + h * DH
                for tb in range(N_CHUNK):
                    ps = proj_ps.tile([128, CHUNK], F32, tag="ps")
                    for ko in range(KO):
                        nc.tensor.matmul(
                            ps,
                            lhsT=wt_sb[:, ko, col:col + DH],
                            rhs=xt[tb][:, ko, :],
                            start=(ko == 0),
                            stop=(ko == KO - 1),
                        )
                    nc.vector.tensor_scalar_add(
                        dst[:, h, tb * CHUNK:(tb + 1) * CHUNK], ps, bqk[qk * HPC + h]
                    )
        # V natural: out [128 tok, 2*dh], lhsT = x^T chunk, rhs = W_v^T
        for tb in range(SC // 128):
            ps = proj_ps.tile([128, CHUNK], F32, tag="ps")
            psv = ps[:, :HPC * DH]
            for ko in range(KO):
                nc.tensor.matmul(
                    psv,
                    lhsT=xt[tb // 4][:, ko, (tb % 4) * 128:(tb % 4 + 1) * 128],
                    rhs=wt_sb[:, ko, 2 * HPC * DH:],
                    start=(ko == 0),
                    stop=(ko == KO - 1),
                )
            nc.vector.tensor_add(
                v_sb[:, :, tb, 0:DH],
                psv.rearrange("p (h d) -> p h d", h=HPC),
                bv_rep.rearrange("p (h d) -> p h d", h=HPC),
            )

    # ---- attention ----
    QB = 512  # query block width
    NQB = SC // QB
    NKT = SC // 128  # 16 key tiles
    st_ps = ctx.enter_context(tc.tile_pool(name="st_ps", bufs=2, space="PSUM"))
    pv_ps = ctx.enter_context(tc.tile_pool(name="pv_ps", bufs=4, space="PSUM"))
    atp = ctx.enter_context(tc.tile_pool(name="atp", bufs=3))
    outp = ctx.enter_context(tc.tile_pool(name="outp", bufs=2))
    rcp = ctx.enter_context(tc.tile_pool(name="rcp", bufs=8))

    for h in range(HPC):
        for qb in range(NQB):
            qs = slice(qb * QB, (qb + 1) * QB)
            pvs = [pv_ps.tile([128, DH + 1], F32, tag="pv", name=f"pv{j}") for j in range(QB // 128)]
            for kt2 in range(NKT // 2):
                st = st_ps.tile([128, 2, QB], F32, tag="st")
                for i in range(2):
                    kt = kt2 * 2 + i
                    nc.tensor.matmul(
                        st[:, i, :],
                        lhsT=kT[:, h, kt * 128:(kt + 1) * 128],
                        rhs=qT[:, h, qs],
                        start=True,
                        stop=True,
                    )
                at = atp.tile([128, 2, QB], BF16, tag="at")
                nc.scalar.activation(at, st, mybir.ActivationFunctionType.Exp, scale=SCALE)
                for i in range(2):
                    kt = kt2 * 2 + i
                    for j in range(QB // 128):
                        nc.tensor.matmul(
                            pvs[j],
                            lhsT=at[:, i, j * 128:(j + 1) * 128],
                            rhs=v_sb[:, h, kt, :],
                            start=(kt == 0),
                            stop=(kt == NKT - 1),
                        )
            ot = outp.tile([128, QB // 128, DH], F32, tag="ot")
            for j in range(QB // 128):
                rc = rcp.tile([128, 1], F32, tag="rc")
                nc.vector.reciprocal(rc, pvs[j][:, DH:DH + 1])
                nc.vector.tensor_scalar_mul(ot[:, j, :], pvs[j][:, 0:DH], rc)
            nc.sync.dma_start(
                out[qs, h * DH:(h + 1) * DH].rearrange("(j p) c -> p j c", p=128),
                ot,
            )


def build_program():
    nc = bacc.Bacc("TRN2", target_bir_lowering=False, debug=False)
    x = nc.dram_tensor("x", [SC, D], BF16, kind="ExternalInput").ap()
    wt = nc.dram_tensor("wt", [D, WCOLS], F32, kind="ExternalInput").ap()
    bias = nc.dram_tensor("bias", [WCOLS], F32, kind="ExternalInput").ap()
    out = nc.dram_tensor("out", [SC, HPC * DH], F32, kind="ExternalOutput").ap()
    with tile.TileContext(nc) as tc:
        with ExitStack() as ctx:
            _mha_body(ctx, tc, out, x, wt, bias)
    nc.compile()
    return nc


_NC = None


def _get_nc():
    global _NC
    if _NC is None:
        _NC = build_program()
    return _NC


def make_in_maps(x, W_qkv, b_qkv):
    import ml_dtypes

    x = np.asarray(x, dtype=np.float32)
    W = np.asarray(W_qkv, dtype=np.float32)
    b = np.asarray(b_qkv, dtype=np.float32)
    x_bf = x.astype(ml_dtypes.bfloat16)
    in_maps = []
    for c in range(N_CORES):
        bsel = c // 4
        h0 = HPC * (c % 4)
        rows = np.concatenate(
            [qkv * D + np.arange(h0 * DH, (h0 + HPC) * DH) for qkv in range(3)]
        )
        Wc = W[rows]  # [768, 1024]
        in_maps.append(
            {
                "x": np.ascontiguousarray(x_bf[bsel]),
                "wt": np.ascontiguousarray(Wc.T),
                "bias": np.ascontiguousarray(b[rows]),
            }
        )
    return in_maps


def gather_output(results):
    outp = np.empty((B, S, D), np.float32)
    for c in range(N_CORES):
        o = results[c]["out"]
        bsel = c // 4
        h0 = HPC * (c % 4)
        outp[bsel, :, h0 * DH:(h0 + HPC) * DH] = o
    return outp


def kernel(x, W_qkv, b_qkv, **run_kwargs):
    in_maps = make_in_maps(x, W_qkv, b_qkv)
    res = run_bass_kernel_spmd(_get_nc(), in_maps, core_ids=list(range(N_CORES)), **run_kwargs)
    out = gather_output(res.results)
    if run_kwargs:
        kernel.last_result = res
    return out
